# revision 9
# baseline (speedup 1.0000x reference)
"""DETR matcher kernel for Trainium2, sharded over 8 NeuronCores.

Reference semantics (nn_DETRMatcher):
    out_prob  = softmax(outputs_class.reshape(bs, -1))
    cost_class= -out_prob[:, zeros(bs)]          # constant along columns
    cost_bbox = cdist_l1(out_bbox, tgt_bbox)
    C         = cost_class + 5*cost_bbox
    rows      = argmin(C, axis=1)
    matched   = zeros.at[ar, rows].set(0)        # writes 0 into zeros -> all zeros
    tgt_idx   = zeros.at[ar, rows].set(0)        # all zeros
    return matched, tgt_idx, out_bbox, tgt_bbox

The two index outputs are identically zero regardless of the data (the
scatter writes the value 0 into zero-initialized arrays), so the
softmax / gather / cdist / argmin pipeline is dead code with respect to
the outputs.  What remains is pure data movement:
    out_bbox = outputs_coord.reshape(bs, Q*4)
    tgt_bbox = target_boxes
Each tensor is bs*Q*4 = 3,240,000 f32 = 12.96 MB; that flat element
count divides by 8 exactly, so the kernel shards it 1-D across the 8
cores (405,000 elements per core per tensor, no padding) and streams
both shards through each core with DRAM->DRAM DMA — the
memory-roofline-minimal device program for this output set
(~3.25 MB payload per core, ~9 us at the 360 GB/s DMA-bus rate).
The integer zero outputs are constants and are materialized host-side.
"""

import os

import numpy as np

import concourse.bass as bass
import concourse.mybir as mybir
from concourse import bass_utils

N_CORES = 8
BS = 900
Q = 900
QD = Q * 4                    # 3600
FLAT = BS * QD                # 3,240,000 elements per tensor
SHARD = FLAT // N_CORES       # 405,000 elements per core
SROWS = 125                   # shard viewed as [125, 3240] for a 2-D AP
SCOLS = SHARD // SROWS        # 3240

# Row-chunks per tensor copy (separate in-flight DMA instructions).
N_CHUNKS = 2

# test.py hooks: set TRACE=True before calling kernel() to request NTFF
# profiling; the BassKernelResults of the last run is stashed here.
TRACE = False
LAST_RESULTS = None

_CACHE = {}


def _scrub_debug(obj):
    # Drop frame-capture strings (absolute file paths, caller tracebacks)
    # from the serialized BIR so its bytes — and therefore the HLO-derived
    # neuron compile-cache key — do not depend on where this file lives or
    # who called us.
    if isinstance(obj, dict):
        for k, v in obj.items():
            if k == "ant_traceback" and isinstance(v, str):
                obj[k] = ""
            elif k == "filename" and isinstance(v, str):
                obj[k] = "<k>"
            elif k == "lineno" and isinstance(v, int):
                obj[k] = 0
            else:
                _scrub_debug(v)
    elif isinstance(obj, list):
        for v in obj:
            _scrub_debug(v)


def _canonical_json_bytes(nc):
    import orjson

    d = orjson.loads(type(nc).to_json_bytes(nc))
    _scrub_debug(d)
    return orjson.dumps(d)


def _build_nc():
    nc = bass.Bass(
        "TRN2", target_bir_lowering=False, debug=False, num_devices=N_CORES
    )
    coord_in = nc.dram_tensor(
        "coord_in", [SROWS, SCOLS], mybir.dt.float32, kind="ExternalInput"
    )
    boxes_in = nc.dram_tensor(
        "boxes_in", [SROWS, SCOLS], mybir.dt.float32, kind="ExternalInput"
    )
    coord_out = nc.dram_tensor(
        "coord_out", [SROWS, SCOLS], mybir.dt.float32, kind="ExternalOutput"
    )
    boxes_out = nc.dram_tensor(
        "boxes_out", [SROWS, SCOLS], mybir.dt.float32, kind="ExternalOutput"
    )

    # Raw Bass: the whole program is N async DRAM->DRAM DMAs plus one
    # final wait for their completion sem.  (Tile's drain/barrier tail is
    # both overkill here and trips the compiler's sync-wait limit.)
    with nc.Block() as block, nc.semaphore("dma_sem") as dma_sem:

        @block.sync
        def _(sync):
            n = 0
            step = (SROWS + N_CHUNKS - 1) // N_CHUNKS
            for src, dst in ((coord_in, coord_out), (boxes_in, boxes_out)):
                for r0 in range(0, SROWS, step):
                    r1 = min(SROWS, r0 + step)
                    sync.dma_start(dst[r0:r1, :], src[r0:r1, :]).then_inc(
                        dma_sem, 16
                    )
                    n += 1
            sync.wait_ge(dma_sem, 16 * n)

    nc.to_json_bytes = lambda: _canonical_json_bytes(nc)
    return nc


def kernel(**inputs):
    coord = np.ascontiguousarray(
        np.asarray(inputs["outputs_coord"], dtype=np.float32)
    ).reshape(N_CORES, SROWS, SCOLS)
    boxes = np.ascontiguousarray(
        np.asarray(inputs["target_boxes"], dtype=np.float32)
    ).reshape(N_CORES, SROWS, SCOLS)
    # Mirrors the harness's x64 mode: with jax x64 off the reference's
    # int64 outputs are silently int32; target_classes carries the same
    # downcast, so its dtype tells us which one the reference produced.
    int_dtype = np.asarray(inputs["target_classes"]).dtype

    if "nc" not in _CACHE:
        _CACHE["nc"] = _build_nc()
    nc = _CACHE["nc"]

    in_maps = [
        {"coord_in": coord[c], "boxes_in": boxes[c]} for c in range(N_CORES)
    ]
    try:
        res = bass_utils.run_bass_kernel_spmd(
            nc, in_maps, core_ids=list(range(N_CORES)), trace=TRACE
        )
    except (ImportError, ModuleNotFoundError):
        # Tracing was requested (TRACE or BASS_TRACE env) but the axon
        # NTFF hook module is absent in this container; rerun untraced.
        os.environ["BASS_NEVER_TRACE"] = "1"
        try:
            res = bass_utils.run_bass_kernel_spmd(
                nc, in_maps, core_ids=list(range(N_CORES)), trace=False
            )
        finally:
            os.environ.pop("BASS_NEVER_TRACE", None)
    global LAST_RESULTS
    LAST_RESULTS = res

    out_bbox = np.concatenate(
        [res.results[c]["coord_out"].reshape(-1) for c in range(N_CORES)]
    ).reshape(BS, QD)
    tgt_bbox = np.concatenate(
        [res.results[c]["boxes_out"].reshape(-1) for c in range(N_CORES)]
    ).reshape(BS, QD)

    matched_indices = np.zeros((BS, Q), dtype=int_dtype)
    tgt_idx = np.zeros((BS, Q), dtype=int_dtype)
    return matched_indices, tgt_idx, out_bbox, tgt_bbox


# revision 11
# speedup vs baseline: 1.0252x; 1.0252x over previous
"""DETR matcher kernel for Trainium2, sharded over 8 NeuronCores.

Reference semantics (nn_DETRMatcher):
    out_prob  = softmax(outputs_class.reshape(bs, -1))
    cost_class= -out_prob[:, zeros(bs)]          # constant along columns
    cost_bbox = cdist_l1(out_bbox, tgt_bbox)
    C         = cost_class + 5*cost_bbox
    rows      = argmin(C, axis=1)
    matched   = zeros.at[ar, rows].set(0)        # writes 0 into zeros -> all zeros
    tgt_idx   = zeros.at[ar, rows].set(0)        # all zeros
    return matched, tgt_idx, out_bbox, tgt_bbox

The two index outputs are identically zero regardless of the data (the
scatter writes the value 0 into zero-initialized arrays), so the
softmax / gather / cdist / argmin pipeline is dead code with respect to
the outputs.  What remains is pure data movement:
    out_bbox = outputs_coord.reshape(bs, Q*4)
    tgt_bbox = target_boxes
Each tensor is bs*Q*4 = 3,240,000 f32 = 12.96 MB; that flat element
count divides by 8 exactly, so the kernel shards it 1-D across the 8
cores (405,000 elements per core per tensor, no padding) and streams
both shards through each core with DRAM->DRAM DMA — the
memory-roofline-minimal device program for this output set
(~3.25 MB payload per core, ~9 us at the 360 GB/s DMA-bus rate).
The integer zero outputs are constants and are materialized host-side.
"""

import os

import numpy as np

import concourse.bass as bass
import concourse.mybir as mybir
from concourse import bass_utils

N_CORES = 8
BS = 900
Q = 900
QD = Q * 4                    # 3600
FLAT = BS * QD                # 3,240,000 elements per tensor
SHARD = FLAT // N_CORES       # 405,000 elements per core
SROWS = 125                   # shard viewed as [125, 3240] for a 2-D AP
SCOLS = SHARD // SROWS        # 3240

# Row-chunks per tensor copy (separate in-flight DMA instructions).
N_CHUNKS = 2

# test.py hooks: set TRACE=True before calling kernel() to request NTFF
# profiling; the BassKernelResults of the last run is stashed here.
TRACE = False
LAST_RESULTS = None

_CACHE = {}


def _scrub_debug(obj):
    # Drop frame-capture strings (absolute file paths, caller tracebacks)
    # from the serialized BIR so its bytes — and therefore the HLO-derived
    # neuron compile-cache key — do not depend on where this file lives or
    # who called us.
    if isinstance(obj, dict):
        for k, v in obj.items():
            if k == "ant_traceback" and isinstance(v, str):
                obj[k] = ""
            elif k == "filename" and isinstance(v, str):
                obj[k] = "<k>"
            elif k == "lineno" and isinstance(v, int):
                obj[k] = 0
            else:
                _scrub_debug(v)
    elif isinstance(obj, list):
        for v in obj:
            _scrub_debug(v)


def _canonical_json_bytes(nc):
    import orjson

    d = orjson.loads(type(nc).to_json_bytes(nc))
    _scrub_debug(d)
    return orjson.dumps(d)


def _build_nc():
    nc = bass.Bass(
        "TRN2", target_bir_lowering=False, debug=False, num_devices=N_CORES
    )
    coord_in = nc.dram_tensor(
        "coord_in", [SROWS, SCOLS], mybir.dt.float32, kind="ExternalInput"
    )
    boxes_in = nc.dram_tensor(
        "boxes_in", [SROWS, SCOLS], mybir.dt.float32, kind="ExternalInput"
    )
    coord_out = nc.dram_tensor(
        "coord_out", [SROWS, SCOLS], mybir.dt.float32, kind="ExternalOutput"
    )
    boxes_out = nc.dram_tensor(
        "boxes_out", [SROWS, SCOLS], mybir.dt.float32, kind="ExternalOutput"
    )

    # Raw Bass: the whole program is N async DRAM->DRAM DMAs.  Each DMA
    # carries a completion-sem update because HWDGE codegen requires sync
    # info ("DGE must have sync info"), but no engine waits on it:
    # Block.__exit__ emits a per-engine InstDrain + barrier tail, and
    # SP's drain retires its outstanding DMAs before the NEFF ends (the
    # same mechanism every Tile kernel's tail relies on for output
    # integrity).  An explicit wait_ge would add a wait dispatch plus
    # the DMA-sem propagation latency to the critical path for nothing.
    with nc.Block() as block, nc.semaphore("dma_sem") as dma_sem:

        @block.sync
        def _(sync):
            step = (SROWS + N_CHUNKS - 1) // N_CHUNKS
            for src, dst in ((coord_in, coord_out), (boxes_in, boxes_out)):
                for r0 in range(0, SROWS, step):
                    r1 = min(SROWS, r0 + step)
                    sync.dma_start(dst[r0:r1, :], src[r0:r1, :]).then_inc(
                        dma_sem, 16
                    )

    nc.to_json_bytes = lambda: _canonical_json_bytes(nc)
    return nc


def kernel(**inputs):
    coord = np.ascontiguousarray(
        np.asarray(inputs["outputs_coord"], dtype=np.float32)
    ).reshape(N_CORES, SROWS, SCOLS)
    boxes = np.ascontiguousarray(
        np.asarray(inputs["target_boxes"], dtype=np.float32)
    ).reshape(N_CORES, SROWS, SCOLS)
    # Mirrors the harness's x64 mode: with jax x64 off the reference's
    # int64 outputs are silently int32; target_classes carries the same
    # downcast, so its dtype tells us which one the reference produced.
    int_dtype = np.asarray(inputs["target_classes"]).dtype

    if "nc" not in _CACHE:
        _CACHE["nc"] = _build_nc()
    nc = _CACHE["nc"]

    in_maps = [
        {"coord_in": coord[c], "boxes_in": boxes[c]} for c in range(N_CORES)
    ]
    try:
        res = bass_utils.run_bass_kernel_spmd(
            nc, in_maps, core_ids=list(range(N_CORES)), trace=TRACE
        )
    except (ImportError, ModuleNotFoundError):
        # Tracing was requested (TRACE or BASS_TRACE env) but the axon
        # NTFF hook module is absent in this container; rerun untraced.
        os.environ["BASS_NEVER_TRACE"] = "1"
        try:
            res = bass_utils.run_bass_kernel_spmd(
                nc, in_maps, core_ids=list(range(N_CORES)), trace=False
            )
        finally:
            os.environ.pop("BASS_NEVER_TRACE", None)
    global LAST_RESULTS
    LAST_RESULTS = res

    out_bbox = np.concatenate(
        [res.results[c]["coord_out"].reshape(-1) for c in range(N_CORES)]
    ).reshape(BS, QD)
    tgt_bbox = np.concatenate(
        [res.results[c]["boxes_out"].reshape(-1) for c in range(N_CORES)]
    ).reshape(BS, QD)

    matched_indices = np.zeros((BS, Q), dtype=int_dtype)
    tgt_idx = np.zeros((BS, Q), dtype=int_dtype)
    return matched_indices, tgt_idx, out_bbox, tgt_bbox


# revision 12
# speedup vs baseline: 1.0904x; 1.0637x over previous
"""DETR matcher kernel for Trainium2, sharded over 8 NeuronCores.

Reference semantics (nn_DETRMatcher):
    out_prob  = softmax(outputs_class.reshape(bs, -1))
    cost_class= -out_prob[:, zeros(bs)]          # constant along columns
    cost_bbox = cdist_l1(out_bbox, tgt_bbox)
    C         = cost_class + 5*cost_bbox
    rows      = argmin(C, axis=1)
    matched   = zeros.at[ar, rows].set(0)        # writes 0 into zeros -> all zeros
    tgt_idx   = zeros.at[ar, rows].set(0)        # all zeros
    return matched, tgt_idx, out_bbox, tgt_bbox

The two index outputs are identically zero regardless of the data (the
scatter writes the value 0 into zero-initialized arrays), so the
softmax / gather / cdist / argmin pipeline is dead code with respect to
the outputs.  What remains is pure data movement:
    out_bbox = outputs_coord.reshape(bs, Q*4)
    tgt_bbox = target_boxes
Each tensor is bs*Q*4 = 3,240,000 f32 = 12.96 MB; that flat element
count divides by 8 exactly, so the kernel shards it 1-D across the 8
cores (405,000 elements per core per tensor, no padding) and streams
both shards through each core with DRAM->DRAM DMA — the
memory-roofline-minimal device program for this output set
(~3.25 MB payload per core, ~9 us at the 360 GB/s DMA-bus rate).
The integer zero outputs are constants and are materialized host-side.
"""

import os

import numpy as np

import concourse.bass as bass
import concourse.mybir as mybir
from concourse import bass_utils

N_CORES = 8
BS = 900
Q = 900
QD = Q * 4                    # 3600
FLAT = BS * QD                # 3,240,000 elements per tensor
SHARD = FLAT // N_CORES       # 405,000 elements per core
SROWS = 125                   # shard viewed as [125, 3240] for a 2-D AP
SCOLS = SHARD // SROWS        # 3240

# Row-chunks per tensor copy (separate in-flight DMA instructions).
N_CHUNKS = 2

# test.py hooks: set TRACE=True before calling kernel() to request NTFF
# profiling; the BassKernelResults of the last run is stashed here.
TRACE = False
LAST_RESULTS = None

_CACHE = {}


def _scrub_debug(obj):
    # Drop frame-capture strings (absolute file paths, caller tracebacks)
    # from the serialized BIR so its bytes — and therefore the HLO-derived
    # neuron compile-cache key — do not depend on where this file lives or
    # who called us.
    if isinstance(obj, dict):
        for k, v in obj.items():
            if k == "ant_traceback" and isinstance(v, str):
                obj[k] = ""
            elif k == "filename" and isinstance(v, str):
                obj[k] = "<k>"
            elif k == "lineno" and isinstance(v, int):
                obj[k] = 0
            else:
                _scrub_debug(v)
    elif isinstance(obj, list):
        for v in obj:
            _scrub_debug(v)


def _canonical_json_bytes(nc):
    import orjson

    d = orjson.loads(type(nc).to_json_bytes(nc))
    _scrub_debug(d)
    return orjson.dumps(d)


def _build_nc():
    nc = bass.Bass(
        "TRN2", target_bir_lowering=False, debug=False, num_devices=N_CORES
    )
    coord_in = nc.dram_tensor(
        "coord_in", [SROWS, SCOLS], mybir.dt.float32, kind="ExternalInput"
    )
    boxes_in = nc.dram_tensor(
        "boxes_in", [SROWS, SCOLS], mybir.dt.float32, kind="ExternalInput"
    )
    coord_out = nc.dram_tensor(
        "coord_out", [SROWS, SCOLS], mybir.dt.float32, kind="ExternalOutput"
    )
    boxes_out = nc.dram_tensor(
        "boxes_out", [SROWS, SCOLS], mybir.dt.float32, kind="ExternalOutput"
    )

    # Raw Bass: the whole program is N async DRAM->DRAM DMAs.  Each DMA
    # carries a completion-sem update because HWDGE codegen requires sync
    # info ("DGE must have sync info"), but no engine waits on it:
    # Block.__exit__ emits a per-engine InstDrain + barrier tail, and
    # SP's drain retires its outstanding DMAs before the NEFF ends (the
    # same mechanism every Tile kernel's tail relies on for output
    # integrity).  An explicit wait_ge would add a wait dispatch plus
    # the DMA-sem propagation latency to the critical path for nothing.
    with nc.Block() as block, nc.semaphore("dma_sem") as dma_sem:

        @block.sync
        def _(sync):
            step = (SROWS + N_CHUNKS - 1) // N_CHUNKS
            for src, dst in ((coord_in, coord_out), (boxes_in, boxes_out)):
                for r0 in range(0, SROWS, step):
                    r1 = min(SROWS, r0 + step)
                    sync.dma_start(dst[r0:r1, :], src[r0:r1, :]).then_inc(
                        dma_sem, 16
                    )

    # Strip the Bass preamble's const-table memsets and the initial
    # all-engine barrier from the entry block (~0.7 us): this program
    # reads none of the const SBUF tensors, and with a single working
    # engine there is nothing for the start barrier to order.  Only the
    # first basic block is touched — the Block-exit drain+barrier tail,
    # which retires the in-flight DMAs, lives in its own block and stays.
    main_bb = nc.m.functions[0].blocks[0]
    for ins in [
        i
        for i in main_bb.instructions
        if type(i).__name__ in ("InstMemset", "InstDrain", "InstEventSemaphore")
    ]:
        main_bb.instructions.remove(ins)

    nc.to_json_bytes = lambda: _canonical_json_bytes(nc)
    return nc


def kernel(**inputs):
    coord = np.ascontiguousarray(
        np.asarray(inputs["outputs_coord"], dtype=np.float32)
    ).reshape(N_CORES, SROWS, SCOLS)
    boxes = np.ascontiguousarray(
        np.asarray(inputs["target_boxes"], dtype=np.float32)
    ).reshape(N_CORES, SROWS, SCOLS)
    # Mirrors the harness's x64 mode: with jax x64 off the reference's
    # int64 outputs are silently int32; target_classes carries the same
    # downcast, so its dtype tells us which one the reference produced.
    int_dtype = np.asarray(inputs["target_classes"]).dtype

    if "nc" not in _CACHE:
        _CACHE["nc"] = _build_nc()
    nc = _CACHE["nc"]

    in_maps = [
        {"coord_in": coord[c], "boxes_in": boxes[c]} for c in range(N_CORES)
    ]
    try:
        res = bass_utils.run_bass_kernel_spmd(
            nc, in_maps, core_ids=list(range(N_CORES)), trace=TRACE
        )
    except (ImportError, ModuleNotFoundError):
        # Tracing was requested (TRACE or BASS_TRACE env) but the axon
        # NTFF hook module is absent in this container; rerun untraced.
        os.environ["BASS_NEVER_TRACE"] = "1"
        try:
            res = bass_utils.run_bass_kernel_spmd(
                nc, in_maps, core_ids=list(range(N_CORES)), trace=False
            )
        finally:
            os.environ.pop("BASS_NEVER_TRACE", None)
    global LAST_RESULTS
    LAST_RESULTS = res

    out_bbox = np.concatenate(
        [res.results[c]["coord_out"].reshape(-1) for c in range(N_CORES)]
    ).reshape(BS, QD)
    tgt_bbox = np.concatenate(
        [res.results[c]["boxes_out"].reshape(-1) for c in range(N_CORES)]
    ).reshape(BS, QD)

    matched_indices = np.zeros((BS, Q), dtype=int_dtype)
    tgt_idx = np.zeros((BS, Q), dtype=int_dtype)
    return matched_indices, tgt_idx, out_bbox, tgt_bbox


# revision 13
# speedup vs baseline: 1.0952x; 1.0044x over previous
"""DETR matcher kernel for Trainium2, sharded over 8 NeuronCores.

Reference semantics (nn_DETRMatcher):
    out_prob  = softmax(outputs_class.reshape(bs, -1))
    cost_class= -out_prob[:, zeros(bs)]          # constant along columns
    cost_bbox = cdist_l1(out_bbox, tgt_bbox)
    C         = cost_class + 5*cost_bbox
    rows      = argmin(C, axis=1)
    matched   = zeros.at[ar, rows].set(0)        # writes 0 into zeros -> all zeros
    tgt_idx   = zeros.at[ar, rows].set(0)        # all zeros
    return matched, tgt_idx, out_bbox, tgt_bbox

The two index outputs are identically zero regardless of the data (the
scatter writes the value 0 into zero-initialized arrays), so the
softmax / gather / cdist / argmin pipeline is dead code with respect to
the outputs.  What remains is pure data movement:
    out_bbox = outputs_coord.reshape(bs, Q*4)
    tgt_bbox = target_boxes
Each tensor is bs*Q*4 = 3,240,000 f32 = 12.96 MB; that flat element
count divides by 8 exactly, so the kernel shards it 1-D across the 8
cores (405,000 elements per core per tensor, no padding) and streams
both shards through each core with DRAM->DRAM DMA — the
memory-roofline-minimal device program for this output set
(~3.25 MB payload per core, ~9 us at the 360 GB/s DMA-bus rate).
The integer zero outputs are constants and are materialized host-side.
"""

import os

import numpy as np

import concourse.bass as bass
import concourse.mybir as mybir
from concourse import bass_utils

N_CORES = 8
BS = 900
Q = 900
QD = Q * 4                    # 3600
FLAT = BS * QD                # 3,240,000 elements per tensor
SHARD = FLAT // N_CORES       # 405,000 elements per core
SROWS = 125                   # shard viewed as [125, 3240] for a 2-D AP
SCOLS = SHARD // SROWS        # 3240

# Row-chunks per tensor copy (separate in-flight DMA instructions).
N_CHUNKS = 2

# test.py hooks: set TRACE=True before calling kernel() to request NTFF
# profiling; the BassKernelResults of the last run is stashed here.
TRACE = False
LAST_RESULTS = None

_CACHE = {}


def _scrub_debug(obj):
    # Drop frame-capture strings (absolute file paths, caller tracebacks)
    # from the serialized BIR so its bytes — and therefore the HLO-derived
    # neuron compile-cache key — do not depend on where this file lives or
    # who called us.
    if isinstance(obj, dict):
        for k, v in obj.items():
            if k == "ant_traceback" and isinstance(v, str):
                obj[k] = ""
            elif k == "filename" and isinstance(v, str):
                obj[k] = "<k>"
            elif k == "lineno" and isinstance(v, int):
                obj[k] = 0
            else:
                _scrub_debug(v)
    elif isinstance(obj, list):
        for v in obj:
            _scrub_debug(v)


def _canonical_json_bytes(nc):
    import orjson

    d = orjson.loads(type(nc).to_json_bytes(nc))
    _scrub_debug(d)
    return orjson.dumps(d)


def _build_nc():
    nc = bass.Bass(
        "TRN2", target_bir_lowering=False, debug=False, num_devices=N_CORES
    )
    coord_in = nc.dram_tensor(
        "coord_in", [SROWS, SCOLS], mybir.dt.float32, kind="ExternalInput"
    )
    boxes_in = nc.dram_tensor(
        "boxes_in", [SROWS, SCOLS], mybir.dt.float32, kind="ExternalInput"
    )
    coord_out = nc.dram_tensor(
        "coord_out", [SROWS, SCOLS], mybir.dt.float32, kind="ExternalOutput"
    )
    boxes_out = nc.dram_tensor(
        "boxes_out", [SROWS, SCOLS], mybir.dt.float32, kind="ExternalOutput"
    )

    # Raw Bass, flat (no nc.Block, so no per-engine branch hops): the
    # program is N async DRAM->DRAM DMAs followed by a drain+barrier
    # tail.  Each DMA carries a completion-sem update because HWDGE
    # codegen requires sync info ("DGE must have sync info"), but no
    # engine waits on it: SP's InstDrain in the tail retires its
    # outstanding DMAs before the NEFF ends (the same mechanism every
    # Tile kernel's tail relies on for output integrity).  An explicit
    # wait_ge would add a wait dispatch plus the DMA-sem propagation
    # latency to the critical path for nothing.
    with nc.semaphore("dma_sem") as dma_sem:
        step = (SROWS + N_CHUNKS - 1) // N_CHUNKS
        for src, dst in ((coord_in, coord_out), (boxes_in, boxes_out)):
            for r0 in range(0, SROWS, step):
                r1 = min(SROWS, r0 + step)
                nc.sync.dma_start(dst[r0:r1, :], src[r0:r1, :]).then_inc(
                    dma_sem, 16
                )
        nc.all_engine_barrier()

    # Strip the Bass preamble's const-table memsets and the initial
    # all-engine barrier (~0.7 us): this program reads none of the const
    # SBUF tensors, and with a single working engine there is nothing
    # for the start barrier to order.  Only instructions BEFORE the
    # first DMA are touched — the drain+barrier tail emitted above,
    # which retires the in-flight DMAs, comes after them and stays.
    main_bb = nc.m.functions[0].blocks[0]
    drop = []
    for ins in main_bb.instructions:
        if type(ins).__name__ == "InstDMACopy":
            break
        if type(ins).__name__ in ("InstMemset", "InstDrain", "InstEventSemaphore"):
            drop.append(ins)
    for ins in drop:
        main_bb.instructions.remove(ins)

    nc.to_json_bytes = lambda: _canonical_json_bytes(nc)
    return nc


def kernel(**inputs):
    coord = np.ascontiguousarray(
        np.asarray(inputs["outputs_coord"], dtype=np.float32)
    ).reshape(N_CORES, SROWS, SCOLS)
    boxes = np.ascontiguousarray(
        np.asarray(inputs["target_boxes"], dtype=np.float32)
    ).reshape(N_CORES, SROWS, SCOLS)
    # Mirrors the harness's x64 mode: with jax x64 off the reference's
    # int64 outputs are silently int32; target_classes carries the same
    # downcast, so its dtype tells us which one the reference produced.
    int_dtype = np.asarray(inputs["target_classes"]).dtype

    if "nc" not in _CACHE:
        _CACHE["nc"] = _build_nc()
    nc = _CACHE["nc"]

    in_maps = [
        {"coord_in": coord[c], "boxes_in": boxes[c]} for c in range(N_CORES)
    ]
    try:
        res = bass_utils.run_bass_kernel_spmd(
            nc, in_maps, core_ids=list(range(N_CORES)), trace=TRACE
        )
    except (ImportError, ModuleNotFoundError):
        # Tracing was requested (TRACE or BASS_TRACE env) but the axon
        # NTFF hook module is absent in this container; rerun untraced.
        os.environ["BASS_NEVER_TRACE"] = "1"
        try:
            res = bass_utils.run_bass_kernel_spmd(
                nc, in_maps, core_ids=list(range(N_CORES)), trace=False
            )
        finally:
            os.environ.pop("BASS_NEVER_TRACE", None)
    global LAST_RESULTS
    LAST_RESULTS = res

    out_bbox = np.concatenate(
        [res.results[c]["coord_out"].reshape(-1) for c in range(N_CORES)]
    ).reshape(BS, QD)
    tgt_bbox = np.concatenate(
        [res.results[c]["boxes_out"].reshape(-1) for c in range(N_CORES)]
    ).reshape(BS, QD)

    matched_indices = np.zeros((BS, Q), dtype=int_dtype)
    tgt_idx = np.zeros((BS, Q), dtype=int_dtype)
    return matched_indices, tgt_idx, out_bbox, tgt_bbox
